# revision 24
# baseline (speedup 1.0000x reference)
"""Trainium2 Bass kernel for nn_ContrastiveLoss (8-core SPMD).

Math (reference): z = row-normalized emb_in [8192,1024]; S = z@z.T / 0.5;
only rows i < n=2048 of S are used:
  denom_i   = sum_k exp(S[i,k]) - exp(S[i,i])
  loss      = sum_i (n-1-i)*log(denom_i) - sum_{i<j<n} S[i,j]
  out       = (-2/n)*(n-1)*loss

Sharding: 2x4-shard the needed S block [2048 x 8192] across 8 cores (core
j owns rows [1024*(j//4), ...) x cols [2048*(j%4), ...)).  fp8e4 DoubleRow
GEMM (qT stationary, kT moving) accumulates in PSUM; exp + per-row sums are
fused into the PSUM drain on ScalarE (activation accum_out, in-place on the
PSUM tile).  Host does the tiny log/weighted combine; the triu term
factorizes exactly: sum_{i<j<n} S_ij = (||sum z_i||^2 - sum ||z_i||^2)/(2T).

Schedule (B9): per-core inputs live in ONE DRAM param packed in planned
arrival order (each chunk [8, n] m-major per partition, so every DMA span
is one contiguous full-bandwidth run per partition).  Matmul stripes gate
on per-span DMA semaphores, so k ships in half-size spans (12 spans total;
HWDGE desc-gens serialize at ~650ns each, so more spans would delay the
q tail).  ibs 0/1 are striped asymmetrically: ib0 starts alone off the
smallest possible first span (q[0:128]+k[0:128]), ib1 catches up with one
512-wide call once its stationary lands; both then stripe k chunks as
they arrive (ScalarE is the critical engine; stripes keep it fed through
the DMA window).  ibs 2-7 use one full-width activation each, double-
buffered across 2 PSUM tiles; dummy matmuls keep the PE p-state clock
alive until real matmuls start.  Measured dead ends are kept as commented
schedule variants below.
"""

import os
import sys
import numpy as np

sys.path.insert(0, "/opt/trn_rl_repo")

import ml_dtypes  # noqa: E402

import concourse.bass as bass  # noqa: E402
import concourse.bacc as bacc  # noqa: E402
import concourse.mybir as mybir  # noqa: E402
from concourse import tile  # noqa: E402
from concourse.bass_utils import run_bass_kernel_spmd  # noqa: E402

B = 8192
D = 1024
N = B // 4          # 2048 rows of S actually used
CORES = 8
R_GROUPS = 2
C_GROUPS = CORES // R_GROUPS
QR = N // R_GROUPS      # 1024 rows per core
KC = B // C_GROUPS      # 2048 cols per core
TEMP_SCALE = 2.0        # 1/temperature
N_IB = QR // 128        # 8 i-blocks per core

_CACHED_NC = None
LAST_RESULTS = None

PSUM_BANK = 512         # f32 elems per PSUM bank

# ---------------------------------------------------------------------------
# Schedule variants.  LAYOUT: chunk arrival order; SPANS: chunks per
# dma_start (None = one each); GENS: (tile_idx, [(ib, klo, khi), ...]) --
# matmuls of all entries, then one ACT per entry (packed at sequential psum
# cols); DUMMIES: (count, width) PE warmup matmuls.

SCHEDULES = {
    "BASELINE": dict(
        LAYOUT=[
            ("q", 0, 256), ("k", 0, 256), ("k", 256, 512), ("k", 512, 1024),
            ("k", 1024, 1536), ("k", 1536, 2048), ("q", 256, 512),
            ("q", 512, 1024),
        ],
        SPANS=[(0, 1), (2, 2), (3, 3), (4, 4), (5, 5), (6, 6), (7, 7)],
        GENS=[
            (0, [(0, 0, 512), (1, 0, 512)]),
            (1, [(0, 512, 1024), (1, 512, 1024)]),
            (0, [(0, 1024, 1536), (1, 1024, 1536)]),
            (1, [(0, 1536, 2048), (1, 1536, 2048)]),
            (0, [(2, 0, 2048)]),
            (1, [(3, 0, 2048)]),
            (0, [(4, 0, 2048)]),
            (1, [(5, 0, 2048)]),
            (0, [(6, 0, 2048)]),
            (1, [(7, 0, 2048)]),
        ],
        DUMMIES=(8, 512),
    ),
    "V1": dict(
        LAYOUT=[
            ("q", 0, 128), ("k", 0, 128), ("q", 128, 256), ("k", 128, 512),
            ("k", 512, 1024), ("q", 256, 512), ("k", 1024, 2048),
            ("q", 512, 1024),
        ],
        SPANS=None,
        GENS=[
            (0, [(0, 0, 128), (1, 0, 128)]),
            (1, [(0, 128, 512), (1, 128, 512)]),
            (0, [(0, 512, 1024), (1, 512, 1024)]),
            (1, [(2, 0, 1024)]),
            (0, [(3, 0, 1024)]),
            (1, [(0, 1024, 2048), (1, 1024, 2048)]),
            (0, [(2, 1024, 2048)]),
            (1, [(3, 1024, 2048)]),
            (0, [(4, 0, 2048)]),
            (1, [(5, 0, 2048)]),
            (0, [(6, 0, 2048)]),
            (1, [(7, 0, 2048)]),
        ],
        DUMMIES=(20, 256),
    ),
    "V2": dict(
        LAYOUT=[
            ("q", 0, 128), ("k", 0, 128), ("q", 128, 256), ("k", 128, 512),
            ("k", 512, 1024), ("q", 256, 512), ("k", 1024, 2048),
            ("q", 512, 1024),
        ],
        SPANS=None,
        GENS=[
            (0, [(0, 0, 128), (1, 0, 128)]),
            (1, [(0, 128, 512), (1, 128, 512)]),
            (0, [(0, 512, 1024), (1, 512, 1024)]),
            (1, [(0, 1024, 2048), (1, 1024, 2048)]),
            (0, [(2, 0, 2048)]),
            (1, [(3, 0, 2048)]),
            (0, [(4, 0, 2048)]),
            (1, [(5, 0, 2048)]),
            (0, [(6, 0, 2048)]),
            (1, [(7, 0, 2048)]),
        ],
        DUMMIES=(20, 256),
    ),
    "V3": dict(
        LAYOUT=[
            ("q", 0, 128), ("k", 0, 128), ("q", 128, 256), ("k", 128, 512),
            ("k", 512, 1024), ("q", 256, 512), ("k", 1024, 2048),
            ("q", 512, 1024),
        ],
        SPANS=None,
        GENS=[
            (0, [(0, 0, 128), (1, 0, 128)]),
            (1, [(0, 128, 512), (1, 128, 512)]),
            (0, [(0, 512, 1024), (1, 512, 1024), (2, 0, 1024)]),
            (1, [(0, 1024, 2048), (1, 1024, 2048)]),
            (0, [(3, 0, 2048)]),
            (1, [(2, 1024, 2048)]),
            (0, [(4, 0, 2048)]),
            (1, [(5, 0, 2048)]),
            (0, [(6, 0, 2048)]),
            (1, [(7, 0, 2048)]),
        ],
        DUMMIES=(20, 256),
    ),
    # V4: bundled tiny first span for an early first exp; k in ~512-col
    # spans so each matmul generation hides inside the previous ACT; last
    # striped gens both on ps1 so ps0 frees early for the first full;
    # q-tail split in two so at most 2 fulls gate on the final span.
    "V4": dict(
        LAYOUT=[
            ("q", 0, 128), ("k", 0, 128), ("q", 128, 256), ("k", 128, 640),
            ("k", 640, 1152), ("q", 256, 512), ("k", 1152, 1664),
            ("k", 1664, 2048), ("q", 512, 768), ("q", 768, 1024),
        ],
        SPANS=[(0, 1), (2, 3), (4, 4), (5, 5), (6, 6), (7, 7), (8, 8), (9, 9)],
        GENS=[
            (0, [(0, 0, 128), (1, 0, 128)]),
            (1, [(0, 128, 640), (1, 128, 640)]),
            (0, [(0, 640, 1152), (1, 640, 1152)]),
            (1, [(0, 1152, 1664), (1, 1152, 1664)]),
            (1, [(0, 1664, 2048), (1, 1664, 2048)]),
            (0, [(2, 0, 2048)]),
            (1, [(3, 0, 2048)]),
            (0, [(4, 0, 2048)]),
            (1, [(5, 0, 2048)]),
            (0, [(6, 0, 2048)]),
            (1, [(7, 0, 2048)]),
        ],
        DUMMIES=(24, 256),
    ),
    # V5: q[0:256] bundled in span0 (both striped ibs unlocked together);
    # finer early k spans; ibs 0/1 striped to 1920 only, their 128-wide
    # tails exp'd after the first full; first full (ib2) lands on ps1 right
    # after its last striped ACT so its matmuls overlap ACTs on ps0.
    "V5": dict(
        LAYOUT=[
            ("q", 0, 256), ("k", 0, 128), ("k", 128, 384), ("k", 384, 896),
            ("k", 896, 1408), ("q", 256, 512), ("k", 1408, 1920),
            ("k", 1920, 2048), ("q", 512, 768), ("q", 768, 1024),
        ],
        SPANS=[(0, 1), (2, 2), (3, 3), (4, 4), (5, 5), (6, 6), (7, 7),
               (8, 8), (9, 9)],
        GENS=[
            (0, [(0, 0, 128), (1, 0, 128)]),
            (1, [(0, 128, 384), (1, 128, 384)]),
            (0, [(0, 384, 896), (1, 384, 896)]),
            (1, [(0, 896, 1408), (1, 896, 1408)]),
            (0, [(0, 1408, 1920), (1, 1408, 1920)]),
            (1, [(2, 0, 2048)]),
            (0, [(0, 1920, 2048), (1, 1920, 2048)]),
            (1, [(3, 0, 2048)]),
            (0, [(4, 0, 2048)]),
            (1, [(5, 0, 2048)]),
            (0, [(6, 0, 2048)]),
            (1, [(7, 0, 2048)]),
        ],
        DUMMIES=(18, 256),
    ),
    # B2: baseline with gen0 split at k=256 so the first exp fires as soon
    # as span0 (q[0:256]+k[0:256]) lands instead of waiting for k[256:512].
    "B2": dict(
        LAYOUT=[
            ("q", 0, 256), ("k", 0, 256), ("k", 256, 512), ("k", 512, 1024),
            ("k", 1024, 1536), ("k", 1536, 2048), ("q", 256, 512),
            ("q", 512, 1024),
        ],
        SPANS=[(0, 1), (2, 2), (3, 3), (4, 4), (5, 5), (6, 6), (7, 7)],
        GENS=[
            (0, [(0, 0, 256), (1, 0, 256)]),
            (1, [(0, 256, 512), (1, 256, 512)]),
            (0, [(0, 512, 1024), (1, 512, 1024)]),
            (1, [(0, 1024, 1536), (1, 1024, 1536)]),
            (0, [(0, 1536, 2048), (1, 1536, 2048)]),
            (1, [(2, 0, 2048)]),
            (0, [(3, 0, 2048)]),
            (1, [(4, 0, 2048)]),
            (0, [(5, 0, 2048)]),
            (1, [(6, 0, 2048)]),
            (0, [(7, 0, 2048)]),
        ],
        DUMMIES=(8, 512),
    ),
    # B3: B2 with a smaller first span (q[0:256]+k[0:128]) and one more
    # dummy so the PE p-state clock covers the earlier first matmuls.
    "B3": dict(
        LAYOUT=[
            ("q", 0, 256), ("k", 0, 128), ("k", 128, 512), ("k", 512, 1024),
            ("k", 1024, 1536), ("k", 1536, 2048), ("q", 256, 512),
            ("q", 512, 1024),
        ],
        SPANS=[(0, 1), (2, 2), (3, 3), (4, 4), (5, 5), (6, 6), (7, 7)],
        GENS=[
            (0, [(0, 0, 128), (1, 0, 128)]),
            (1, [(0, 128, 512), (1, 128, 512)]),
            (0, [(0, 512, 1024), (1, 512, 1024)]),
            (1, [(0, 1024, 1536), (1, 1024, 1536)]),
            (0, [(0, 1536, 2048), (1, 1536, 2048)]),
            (1, [(2, 0, 2048)]),
            (0, [(3, 0, 2048)]),
            (1, [(4, 0, 2048)]),
            (0, [(5, 0, 2048)]),
            (1, [(6, 0, 2048)]),
            (0, [(7, 0, 2048)]),
        ],
        DUMMIES=(9, 512),
    ),
    # B4: B2 with k DMA spans split in half (matmul stripes gate on span
    # semaphores, so finer spans start each generation's matmuls earlier
    # without adding ACT calls) and q[256:384] landing before q[384:512]
    # (the first full i-block's stationary arrives sooner).
    "B4": dict(
        LAYOUT=[
            ("q", 0, 256), ("k", 0, 256), ("k", 256, 512), ("k", 512, 768),
            ("k", 768, 1024), ("k", 1024, 1280), ("k", 1280, 1536),
            ("k", 1536, 1792), ("k", 1792, 2048), ("q", 256, 384),
            ("q", 384, 512), ("q", 512, 1024),
        ],
        SPANS=[(0, 1), (2, 2), (3, 3), (4, 4), (5, 5), (6, 6), (7, 7),
               (8, 8), (9, 9), (10, 10), (11, 11)],
        GENS=[
            (0, [(0, 0, 256), (1, 0, 256)]),
            (1, [(0, 256, 512), (1, 256, 512)]),
            (0, [(0, 512, 1024), (1, 512, 1024)]),
            (1, [(0, 1024, 1536), (1, 1024, 1536)]),
            (0, [(0, 1536, 2048), (1, 1536, 2048)]),
            (1, [(2, 0, 2048)]),
            (0, [(3, 0, 2048)]),
            (1, [(4, 0, 2048)]),
            (0, [(5, 0, 2048)]),
            (1, [(6, 0, 2048)]),
            (0, [(7, 0, 2048)]),
        ],
        DUMMIES=(8, 512),
    ),
    # B5: B4 with the last striped gen moved onto ps1 (so ps0 frees after
    # gen [512:1024]) and the first full (ib2) on ps0 — its 16 matmuls run
    # while the [1024:2048] striped ACTs drain on ps1.
    "B5": dict(
        LAYOUT=[
            ("q", 0, 256), ("k", 0, 256), ("k", 256, 512), ("k", 512, 768),
            ("k", 768, 1024), ("k", 1024, 1280), ("k", 1280, 1536),
            ("k", 1536, 1792), ("k", 1792, 2048), ("q", 256, 384),
            ("q", 384, 512), ("q", 512, 1024),
        ],
        SPANS=[(0, 1), (2, 2), (3, 3), (4, 4), (5, 5), (6, 6), (7, 7),
               (8, 8), (9, 9), (10, 10), (11, 11)],
        GENS=[
            (0, [(0, 0, 256), (1, 0, 256)]),
            (1, [(0, 256, 512), (1, 256, 512)]),
            (0, [(0, 512, 1024), (1, 512, 1024)]),
            (1, [(0, 1024, 1536), (1, 1024, 1536)]),
            (1, [(0, 1536, 2048), (1, 1536, 2048)]),
            (0, [(2, 0, 2048)]),
            (1, [(3, 0, 2048)]),
            (0, [(4, 0, 2048)]),
            (1, [(5, 0, 2048)]),
            (0, [(6, 0, 2048)]),
            (1, [(7, 0, 2048)]),
        ],
        DUMMIES=(8, 512),
    ),
    # B7: B4 with the first k chunk split at 128 so the first exp fires at
    # ~4.3us and the striped ACT chain runs gapless into [512:1024].
    "B7": dict(
        LAYOUT=[
            ("q", 0, 256), ("k", 0, 128), ("k", 128, 256), ("k", 256, 512),
            ("k", 512, 768), ("k", 768, 1024), ("k", 1024, 1280),
            ("k", 1280, 1536), ("k", 1536, 1792), ("k", 1792, 2048),
            ("q", 256, 384), ("q", 384, 512), ("q", 512, 1024),
        ],
        SPANS=[(0, 1), (2, 2), (3, 3), (4, 4), (5, 5), (6, 6), (7, 7),
               (8, 8), (9, 9), (10, 10), (11, 11), (12, 12)],
        GENS=[
            (0, [(0, 0, 128), (1, 0, 128)]),
            (1, [(0, 128, 512), (1, 128, 512)]),
            (0, [(0, 512, 1024), (1, 512, 1024)]),
            (1, [(0, 1024, 1536), (1, 1024, 1536)]),
            (0, [(0, 1536, 2048), (1, 1536, 2048)]),
            (1, [(2, 0, 2048)]),
            (0, [(3, 0, 2048)]),
            (1, [(4, 0, 2048)]),
            (0, [(5, 0, 2048)]),
            (1, [(6, 0, 2048)]),
            (0, [(7, 0, 2048)]),
        ],
        DUMMIES=(8, 512),
    ),
    # B8: B7 with the [128:512] striped gen split at 256 so its first ACT
    # fires off the k[128:256] span instead of waiting for k[256:512].
    "B8": dict(
        LAYOUT=[
            ("q", 0, 256), ("k", 0, 128), ("k", 128, 256), ("k", 256, 512),
            ("k", 512, 768), ("k", 768, 1024), ("k", 1024, 1280),
            ("k", 1280, 1536), ("k", 1536, 1792), ("k", 1792, 2048),
            ("q", 256, 384), ("q", 384, 512), ("q", 512, 1024),
        ],
        SPANS=[(0, 1), (2, 2), (3, 3), (4, 4), (5, 5), (6, 6), (7, 7),
               (8, 8), (9, 9), (10, 10), (11, 11), (12, 12)],
        GENS=[
            (0, [(0, 0, 128), (1, 0, 128)]),
            (1, [(0, 128, 256), (1, 128, 256)]),
            (1, [(0, 256, 512), (1, 256, 512)]),
            (0, [(0, 512, 1024), (1, 512, 1024)]),
            (1, [(0, 1024, 1536), (1, 1024, 1536)]),
            (0, [(0, 1536, 2048), (1, 1536, 2048)]),
            (1, [(2, 0, 2048)]),
            (0, [(3, 0, 2048)]),
            (1, [(4, 0, 2048)]),
            (0, [(5, 0, 2048)]),
            (1, [(6, 0, 2048)]),
            (0, [(7, 0, 2048)]),
        ],
        DUMMIES=(8, 512),
    ),
    # B9: asymmetric early striping -- ib0 starts alone off the smallest
    # possible first span; ib1 joins with one 512-wide call once q[128:256]
    # lands.  Same call count as B7, earlier ACT start, tighter packing.
    "B9": dict(
        LAYOUT=[
            ("q", 0, 128), ("k", 0, 128), ("k", 128, 256), ("k", 256, 512),
            ("q", 128, 256), ("k", 512, 768), ("k", 768, 1024),
            ("k", 1024, 1280), ("k", 1280, 1536), ("k", 1536, 1792),
            ("k", 1792, 2048), ("q", 256, 512), ("q", 512, 1024),
        ],
        SPANS=[(0, 1), (2, 2), (3, 3), (4, 4), (5, 5), (6, 6), (7, 7),
               (8, 8), (9, 9), (10, 10), (11, 11), (12, 12)],
        GENS=[
            (0, [(0, 0, 128)]),
            (1, [(0, 128, 256)]),
            (0, [(0, 256, 512)]),
            (1, [(1, 0, 512)]),
            (0, [(0, 512, 1024), (1, 512, 1024)]),
            (1, [(0, 1024, 1536), (1, 1024, 1536)]),
            (0, [(0, 1536, 2048), (1, 1536, 2048)]),
            (1, [(2, 0, 2048)]),
            (0, [(3, 0, 2048)]),
            (1, [(4, 0, 2048)]),
            (0, [(5, 0, 2048)]),
            (1, [(6, 0, 2048)]),
            (0, [(7, 0, 2048)]),
        ],
        DUMMIES=(7, 512),
    ),
    # B10: B9 with single-ib striped gens -- same ACT call count, but tile
    # hand-offs happen per seg, so each generation's matmuls (mid p-state)
    # expose half the latency into the ACT chain.
    "B10": dict(
        LAYOUT=[
            ("q", 0, 128), ("k", 0, 128), ("k", 128, 256), ("k", 256, 512),
            ("q", 128, 256), ("k", 512, 768), ("k", 768, 1024),
            ("k", 1024, 1280), ("k", 1280, 1536), ("k", 1536, 1792),
            ("k", 1792, 2048), ("q", 256, 512), ("q", 512, 1024),
        ],
        SPANS=[(0, 1), (2, 2), (3, 3), (4, 4), (5, 5), (6, 6), (7, 7),
               (8, 8), (9, 9), (10, 10), (11, 11), (12, 12)],
        GENS=[
            (0, [(0, 0, 128)]),
            (1, [(0, 128, 256)]),
            (0, [(0, 256, 512)]),
            (1, [(1, 0, 512)]),
            (0, [(0, 512, 1024)]),
            (1, [(1, 512, 1024)]),
            (0, [(0, 1024, 1536)]),
            (1, [(1, 1024, 1536)]),
            (0, [(0, 1536, 2048)]),
            (1, [(1, 1536, 2048)]),
            (0, [(2, 0, 2048)]),
            (1, [(3, 0, 2048)]),
            (0, [(4, 0, 2048)]),
            (1, [(5, 0, 2048)]),
            (0, [(6, 0, 2048)]),
            (1, [(7, 0, 2048)]),
        ],
        DUMMIES=(7, 512),
    ),
    # B11: B9 with the [512:1024] striped gen split at 768 across both
    # tiles, filling the ~1us ACT gap after ib1's catch-up call.
    "B11": dict(
        LAYOUT=[
            ("q", 0, 128), ("k", 0, 128), ("k", 128, 256), ("k", 256, 512),
            ("q", 128, 256), ("k", 512, 768), ("k", 768, 1024),
            ("k", 1024, 1280), ("k", 1280, 1536), ("k", 1536, 1792),
            ("k", 1792, 2048), ("q", 256, 512), ("q", 512, 1024),
        ],
        SPANS=[(0, 1), (2, 2), (3, 3), (4, 4), (5, 5), (6, 6), (7, 7),
               (8, 8), (9, 9), (10, 10), (11, 11), (12, 12)],
        GENS=[
            (0, [(0, 0, 128)]),
            (1, [(0, 128, 256)]),
            (0, [(0, 256, 512)]),
            (1, [(1, 0, 512)]),
            (0, [(0, 512, 768), (1, 512, 768)]),
            (1, [(0, 768, 1024), (1, 768, 1024)]),
            (0, [(0, 1024, 1536), (1, 1024, 1536)]),
            (1, [(0, 1536, 2048), (1, 1536, 2048)]),
            (0, [(2, 0, 2048)]),
            (1, [(3, 0, 2048)]),
            (0, [(4, 0, 2048)]),
            (1, [(5, 0, 2048)]),
            (0, [(6, 0, 2048)]),
            (1, [(7, 0, 2048)]),
        ],
        DUMMIES=(7, 512),
    ),
    # B13: B9 with ib0's [1024:1536]+[1536:2048] merged into one 1024-wide
    # call packed with ib1's k-tail on ps0, so ps1's last striped ACT ends
    # ~1us earlier and the first full (ib2) starts sooner; one call fewer.
    "B13": dict(
        LAYOUT=[
            ("q", 0, 128), ("k", 0, 128), ("k", 128, 256), ("k", 256, 512),
            ("q", 128, 256), ("k", 512, 768), ("k", 768, 1024),
            ("k", 1024, 1280), ("k", 1280, 1536), ("k", 1536, 1792),
            ("k", 1792, 2048), ("q", 256, 512), ("q", 512, 1024),
        ],
        SPANS=[(0, 1), (2, 2), (3, 3), (4, 4), (5, 5), (6, 6), (7, 7),
               (8, 8), (9, 9), (10, 10), (11, 11), (12, 12)],
        GENS=[
            (0, [(0, 0, 128)]),
            (1, [(0, 128, 256)]),
            (0, [(0, 256, 512)]),
            (1, [(1, 0, 512)]),
            (0, [(0, 512, 1024), (1, 512, 1024)]),
            (1, [(1, 1024, 1536)]),
            (0, [(0, 1024, 2048), (1, 1536, 2048)]),
            (1, [(2, 0, 2048)]),
            (0, [(3, 0, 2048)]),
            (1, [(4, 0, 2048)]),
            (0, [(5, 0, 2048)]),
            (1, [(6, 0, 2048)]),
            (0, [(7, 0, 2048)]),
        ],
        DUMMIES=(7, 512),
    ),
    # B16: B13 with the merged gen's ACT order flipped (short call first).
    "B16": dict(
        LAYOUT=[
            ("q", 0, 128), ("k", 0, 128), ("k", 128, 256), ("k", 256, 512),
            ("q", 128, 256), ("k", 512, 768), ("k", 768, 1024),
            ("k", 1024, 1280), ("k", 1280, 1536), ("k", 1536, 1792),
            ("k", 1792, 2048), ("q", 256, 512), ("q", 512, 1024),
        ],
        SPANS=[(0, 1), (2, 2), (3, 3), (4, 4), (5, 5), (6, 6), (7, 7),
               (8, 8), (9, 9), (10, 10), (11, 11), (12, 12)],
        GENS=[
            (0, [(0, 0, 128)]),
            (1, [(0, 128, 256)]),
            (0, [(0, 256, 512)]),
            (1, [(1, 0, 512)]),
            (0, [(0, 512, 1024), (1, 512, 1024)]),
            (1, [(1, 1024, 1536)]),
            (0, [(1, 1536, 2048), (0, 1024, 2048)]),
            (1, [(2, 0, 2048)]),
            (0, [(3, 0, 2048)]),
            (1, [(4, 0, 2048)]),
            (0, [(5, 0, 2048)]),
            (1, [(6, 0, 2048)]),
            (0, [(7, 0, 2048)]),
        ],
        DUMMIES=(7, 512),
    ),
    # B17: B9 plus p-state keep-alive filler matmuls before each striped/
    # transition gen, so the PE ramp clock survives the striped phase and
    # the [512:1024]+ matmuls run at full speed instead of mid.
    "B17": dict(
        LAYOUT=[
            ("q", 0, 128), ("k", 0, 128), ("k", 128, 256), ("k", 256, 512),
            ("q", 128, 256), ("k", 512, 768), ("k", 768, 1024),
            ("k", 1024, 1280), ("k", 1280, 1536), ("k", 1536, 1792),
            ("k", 1792, 2048), ("q", 256, 512), ("q", 512, 1024),
        ],
        SPANS=[(0, 1), (2, 2), (3, 3), (4, 4), (5, 5), (6, 6), (7, 7),
               (8, 8), (9, 9), (10, 10), (11, 11), (12, 12)],
        GENS=[
            (0, [(0, 0, 128)]),
            (1, [(0, 128, 256)]),
            (0, [(0, 256, 512)]),
            (1, [(1, 0, 512)]),
            (0, [(0, 512, 1024), (1, 512, 1024)]),
            (1, [(0, 1024, 1536), (1, 1024, 1536)]),
            (0, [(0, 1536, 2048), (1, 1536, 2048)]),
            (1, [(2, 0, 2048)]),
            (0, [(3, 0, 2048)]),
            (1, [(4, 0, 2048)]),
            (0, [(5, 0, 2048)]),
            (1, [(6, 0, 2048)]),
            (0, [(7, 0, 2048)]),
        ],
        DUMMIES=(7, 512),
        FILLERS={1: 2, 2: 2, 3: 2, 4: 2, 5: 2, 6: 2, 7: 2},
    ),
    # B18: B9 with span0 shrunk to q[0:128]+k[0:64] (earlier first exp) and
    # the k tail spans merged to stay within the 12-span desc-gen budget.
    "B18": dict(
        LAYOUT=[
            ("q", 0, 128), ("k", 0, 64), ("k", 64, 128), ("k", 128, 256),
            ("k", 256, 512), ("q", 128, 256), ("k", 512, 768),
            ("k", 768, 1024), ("k", 1024, 1280), ("k", 1280, 1536),
            ("k", 1536, 2048), ("q", 256, 512), ("q", 512, 1024),
        ],
        SPANS=[(0, 1), (2, 2), (3, 3), (4, 4), (5, 5), (6, 6), (7, 7),
               (8, 8), (9, 9), (10, 10), (11, 11), (12, 12)],
        GENS=[
            (0, [(0, 0, 64)]),
            (1, [(0, 64, 128)]),
            (0, [(0, 128, 256)]),
            (1, [(1, 0, 256)]),
            (0, [(0, 256, 512), (1, 256, 512)]),
            (1, [(0, 512, 1024), (1, 512, 1024)]),
            (0, [(0, 1024, 1536), (1, 1024, 1536)]),
            (1, [(0, 1536, 2048), (1, 1536, 2048)]),
            (0, [(2, 0, 2048)]),
            (1, [(3, 0, 2048)]),
            (0, [(4, 0, 2048)]),
            (1, [(5, 0, 2048)]),
            (0, [(6, 0, 2048)]),
            (1, [(7, 0, 2048)]),
        ],
        DUMMIES=(6, 512),
    ),
    # B6: B5 with q[256:384] landing before the k tail so ib2's stationary
    # is ready when ps0 frees.
    "B6": dict(
        LAYOUT=[
            ("q", 0, 256), ("k", 0, 256), ("k", 256, 512), ("k", 512, 768),
            ("k", 768, 1024), ("k", 1024, 1280), ("k", 1280, 1536),
            ("q", 256, 384), ("k", 1536, 1792), ("k", 1792, 2048),
            ("q", 384, 512), ("q", 512, 1024),
        ],
        SPANS=[(0, 1), (2, 2), (3, 3), (4, 4), (5, 5), (6, 6), (7, 7),
               (8, 8), (9, 9), (10, 10), (11, 11)],
        GENS=[
            (0, [(0, 0, 256), (1, 0, 256)]),
            (1, [(0, 256, 512), (1, 256, 512)]),
            (0, [(0, 512, 1024), (1, 512, 1024)]),
            (1, [(0, 1024, 1536), (1, 1024, 1536)]),
            (1, [(0, 1536, 2048), (1, 1536, 2048)]),
            (0, [(2, 0, 2048)]),
            (1, [(3, 0, 2048)]),
            (0, [(4, 0, 2048)]),
            (1, [(5, 0, 2048)]),
            (0, [(6, 0, 2048)]),
            (1, [(7, 0, 2048)]),
        ],
        DUMMIES=(8, 512),
    ),
}

SCHED_NAME = os.environ.get("BASS_SCHED", "B9")
SCHED = SCHEDULES[SCHED_NAME]
LAYOUT = SCHED["LAYOUT"]
DMA_SPANS = SCHED["SPANS"] or [(i, i) for i in range(len(LAYOUT))]
GENS = SCHED["GENS"]
N_DUMMIES, DUMMY_W = SCHED["DUMMIES"]
if "BASS_DUMMIES" in os.environ:
    N_DUMMIES = int(os.environ["BASS_DUMMIES"])
# First DVE_GENS gens skip ACT accum_out (187ns/call read) and row-sum on
# the idle DVE instead (one 3D reduce per gen; its psum read overlaps the
# next gens' ACTs, so tile reuse isn't delayed).
DVE_GENS = int(os.environ.get("BASS_DVE_GENS", SCHED.get("DVE_GENS", 0)))

ACT_SEGS = [(ib, a, b) for _, entries in GENS for (ib, a, b) in entries]
NSEG = len(ACT_SEGS)


def _chunk_offsets():
    offs = []
    off = 0
    for kind, lo, hi in LAYOUT:
        offs.append(off)
        off += 8 * (hi - lo)
    return offs, off


CHUNK_OFF, TOTAL_ELEMS = _chunk_offsets()


def _find_chunk(kind, pos):
    """Chunk index containing element `pos` of q-rows / k-cols."""
    for i, (k, lo, hi) in enumerate(LAYOUT):
        if k == kind and lo <= pos < hi:
            return i
    raise ValueError((kind, pos))


def _mm_stripes(a, b, shift=0):
    """Split k-cols [a,b) at k-chunk boundaries and (shifted) PSUM banks."""
    edges = {a, b}
    for k, lo, hi in LAYOUT:
        if k == "k":
            if a < lo < b:
                edges.add(lo)
            if a < hi < b:
                edges.add(hi)
    x = a + shift
    while True:
        nb = (x // PSUM_BANK + 1) * PSUM_BANK
        if nb >= b + shift:
            break
        edges.add(nb - shift)
        x = nb
    es = sorted(edges)
    return list(zip(es[:-1], es[1:]))


def build_kernel():
    nc = bacc.Bacc("TRN2", target_bir_lowering=False, debug=False)
    f8 = mybir.dt.float8e4
    data = nc.declare_dram_parameter("data", [128, TOTAL_ELEMS], f8, isOutput=False)
    out = nc.declare_dram_parameter("out", [NSEG * 128], mybir.dt.float32, isOutput=True)

    n_c = D // 256  # 4 DoubleRow contraction chunks

    with tile.TileContext(nc) as tc:
        with (
            tc.tile_pool(name="inp", bufs=1) as inp,
            tc.tile_pool(name="work", bufs=1) as work,
            tc.tile_pool(name="acc", bufs=1) as accp,
            tc.tile_pool(name="psum", bufs=1, space="PSUM") as psp,
        ):
            allbuf = inp.tile([128, TOTAL_ELEMS], f8)

            # input DMA issues, in span order (SP queue, first instructions)
            for c0, c1 in DMA_SPANS:
                o0 = CHUNK_OFF[c0]
                o1 = CHUNK_OFF[c1] + 8 * (LAYOUT[c1][2] - LAYOUT[c1][1])
                nc.sync.dma_start(allbuf[:, o0:o1], data[:, o0:o1])

            views = []
            for i, (kind, lo, hi) in enumerate(LAYOUT):
                o = CHUNK_OFF[i]
                views.append(
                    allbuf[:, o:o + 8 * (hi - lo)].rearrange("p (m n) -> p m n", m=8)
                )

            def q_slice(ib, c):
                ci = _find_chunk("q", 128 * ib)
                off = 128 * ib - LAYOUT[ci][1]
                return views[ci][:, 2 * c:2 * c + 2, off:off + 128]

            def k_slice(a, b, c):
                ci = _find_chunk("k", a)
                lo = LAYOUT[ci][1]
                return views[ci][:, 2 * c:2 * c + 2, a - lo:b - lo]

            # PE p-state warmup: dummy matmuls on small memset scratch.
            # The p-state ramp clock starts at the first PE activity and
            # resets on PE idle; these bridge until real matmuls start.
            adum = work.tile([128, 2, 128], f8)
            bdum = work.tile([128, 2, DUMMY_W], f8)
            nc.gpsimd.memset(adum[:], 0.0)
            nc.gpsimd.memset(bdum[:], 0.0)
            ps = [
                psp.tile([128, KC], mybir.dt.float32, name=f"ps{s}") for s in (0, 1)
            ]
            for _ in range(N_DUMMIES):
                nc.tensor.matmul(
                    ps[0][:, 0:DUMMY_W], adum[:], bdum[:], start=True, stop=True,
                    perf_mode=mybir.MatmulPerfMode.DoubleRow,
                )

            exp_acc = accp.tile([128, NSEG], mybir.dt.float32)

            si = 0
            fillers = SCHED.get("FILLERS", {})
            if "BASS_FILL" in os.environ:
                n = int(os.environ["BASS_FILL"])
                fillers = {i: n for i in range(1, 8)} if n else {}
            for gi, (tile_idx, entries) in enumerate(GENS):
                t = ps[tile_idx]
                use_dve = gi < DVE_GENS
                # p-state keep-alive: dummy matmuls sharing this gen's
                # tile-WAR gate; overwritten by the real start=True matmuls.
                for _ in range(fillers.get(gi, 0)):
                    nc.tensor.matmul(
                        t[:, 0:DUMMY_W], adum[:], bdum[:], start=True,
                        stop=True, perf_mode=mybir.MatmulPerfMode.DoubleRow,
                    )
                # all matmuls of the generation, then its ACT drains
                off = 0
                for (ib, klo, khi) in entries:
                    shift = off - klo
                    for (sa, sb) in _mm_stripes(klo, khi, shift):
                        for c in range(n_c):
                            nc.tensor.matmul(
                                t[:, sa + shift:sb + shift],
                                q_slice(ib, c),
                                k_slice(sa, sb, c),
                                start=(c == 0),
                                stop=(c == n_c - 1),
                                perf_mode=mybir.MatmulPerfMode.DoubleRow,
                            )
                    off += khi - klo
                off = 0
                for (ib, klo, khi) in entries:
                    w = khi - klo
                    nc.scalar.activation(
                        t[:, off:off + w],
                        t[:, off:off + w],
                        mybir.ActivationFunctionType.Exp,
                        scale=TEMP_SCALE,
                        accum_out=None if use_dve else exp_acc[:, si:si + 1],
                    )
                    si += 1
                    off += w
                if use_dve:
                    ws = [khi - klo for (_, klo, khi) in entries]
                    assert len(set(ws)) == 1, ws
                    w = ws[0]
                    ng = len(entries)
                    nc.vector.tensor_reduce(
                        exp_acc[:, si - ng:si],
                        t[:, 0:ng * w].rearrange("p (s w) -> p s w", s=ng),
                        mybir.AxisListType.X,
                        mybir.AluOpType.add,
                    )

            # p-major out layout: contiguous per partition, cheap DMA.
            # all-but-last on the idle gpsimd queue, final column on SP.
            out_r = out[:].rearrange("(p a) -> p a", p=128)
            nc.gpsimd.dma_start(out_r[:, 0:NSEG - 1], exp_acc[:, 0:NSEG - 1])
            nc.sync.dma_start(out_r[:, NSEG - 1:NSEG], exp_acc[:, NSEG - 1:NSEG])

    nc.compile()
    return nc


def _get_nc():
    global _CACHED_NC
    if _CACHED_NC is None:
        _CACHED_NC = build_kernel()
    return _CACHED_NC


def _pack_core(zT_f8, r, g):
    """Per-core packed DRAM image [128, TOTAL_ELEMS] following LAYOUT."""
    zq = zT_f8[:, r * QR:(r + 1) * QR]       # [D, QR]
    zk = zT_f8[:, g * KC:(g + 1) * KC]       # [D, KC]
    parts = []
    for kind, lo, hi in LAYOUT:
        src = zq if kind == "q" else zk
        arr = src[:, lo:hi].reshape(128, 8, hi - lo)   # d = p*8 + m
        parts.append(arr.reshape(128, -1))
    return np.ascontiguousarray(np.concatenate(parts, axis=1))


def kernel(emb_in: np.ndarray, **run_kwargs) -> np.ndarray:
    emb = np.asarray(emb_in, dtype=np.float32)
    assert emb.shape == (B, D), emb.shape
    n = N

    # host-side layout prep: normalize rows, transpose to d-major, quantize
    norms = np.sqrt((emb.astype(np.float64) ** 2).sum(axis=1))
    z = emb / norms[:, None].astype(np.float32)
    zT = np.ascontiguousarray(z.T.astype(ml_dtypes.float8_e4m3))  # [D, B]

    in_maps = []
    for j in range(CORES):
        r, g = j // C_GROUPS, j % C_GROUPS
        in_maps.append({"data": _pack_core(zT, r, g)})

    nc = _get_nc()
    res = run_bass_kernel_spmd(nc, in_maps, core_ids=list(range(CORES)), **run_kwargs)
    global LAST_RESULTS
    LAST_RESULTS = res
    outs = [r["out"] for r in res.results]  # per-core exp row-sum partials

    # host combine (tiny): the "all-reduce" of the sharded exp row sums
    expsum = np.zeros(n, dtype=np.float64)
    for j, o in enumerate(outs):
        r = j // C_GROUPS
        o = o.astype(np.float64).reshape(128, NSEG)
        rows = np.zeros((N_IB, 128), dtype=np.float64)
        for si, (ib, a, b) in enumerate(ACT_SEGS):
            rows[ib] += o[:, si]
        expsum[r * QR:(r + 1) * QR] += rows.reshape(-1)
    denom = expsum - np.exp(2.0)
    log_denom = np.log(denom)
    counts = (n - 1) - np.arange(n, dtype=np.float64)

    # triu term, factorized exactly (f64): sum_{i<j<n} z_i.z_j
    zq = z[:n].astype(np.float64)
    s = zq.sum(axis=0)
    cross = (s @ s - (zq * zq).sum()) / 2.0
    sum_sim = TEMP_SCALE * cross

    loss = (counts * log_denom).sum() - sum_sim
    val = (-2.0 / n) * (n - 1) * loss
    return np.asarray(val, dtype=np.float32)


if __name__ == "__main__":
    rng = np.random.default_rng(0)
    x = rng.normal(size=(B, D)).astype(np.float32)
    print(kernel(x))
